# revision 16
# baseline (speedup 1.0000x reference)
import numpy as np

# Problem constants (nn_MyTemporalAttention): x [32, 64, 2048], y [32, 2048, 64]
B, C, L = 32, 64, 2048
KQ = 50
NCORES = 8
BPC = B // NCORES          # batches per core
NM = L // 128              # m-chunks of 128
NL = L // 512              # l-chunks of 512

TRACE = False
LAST_EXEC_NS = None
_cache = {}


def _build(scale: float):
    import concourse.bass as bass
    import concourse.tile as tile
    from concourse import bacc, mybir
    from contextlib import ExitStack

    FP32 = mybir.dt.float32
    BF16 = mybir.dt.bfloat16
    AF = mybir.ActivationFunctionType
    AX = mybir.AxisListType
    OP = mybir.AluOpType
    ts = bass.ts

    nc = bacc.Bacc(
        "TRN2",
        target_bir_lowering=False,
        debug=False,
        enable_asserts=False,
        num_devices=NCORES,
    )
    # x arrives host-augmented with a ones row, pre-cast to bf16: [BPC, 65, L]
    x_d = nc.dram_tensor("x", [BPC, C + 1, L], BF16, kind="ExternalInput").ap()
    # wkq: [65, 128], cols 0-49 = [Wk; bk], cols 64-113 = [Wq; bq], rest zero
    # (q on a 32-aligned partition base so engine ops can address its rows)
    wkq_d = nc.dram_tensor("wkq", [C + 1, 128], BF16, kind="ExternalInput").ap()
    # wv = [Wv; bv] -> [65, 64]
    wv_d = nc.dram_tensor("wv", [C + 1, C], BF16, kind="ExternalInput").ap()
    # y stored transposed per batch: [BPC, C(=64), L]
    y_d = nc.dram_tensor("y", [BPC, C, L], FP32, kind="ExternalOutput").ap()

    with tile.TileContext(nc) as tc, ExitStack() as ctx:
        const = ctx.enter_context(tc.tile_pool(name="const", bufs=1))
        xpool = ctx.enter_context(tc.tile_pool(name="xp", bufs=BPC))
        kqpool = ctx.enter_context(tc.tile_pool(name="kq", bufs=4))
        tmppool = ctx.enter_context(tc.tile_pool(name="tmp", bufs=2))
        vpool = ctx.enter_context(tc.tile_pool(name="v", bufs=2))
        epool = ctx.enter_context(tc.tile_pool(name="e", bufs=6))
        espool = ctx.enter_context(tc.tile_pool(name="es", bufs=2))
        ytpool = ctx.enter_context(tc.tile_pool(name="yt", bufs=2))
        spool = ctx.enter_context(tc.tile_pool(name="stats", bufs=8))
        vppool = ctx.enter_context(tc.tile_pool(name="vp", bufs=4))
        # PSUM: pw = tag pwm [128,1024] (2 banks), bufs=3 -> 6 banks
        #       py = pyt0/pyt1 [128,512] (1 bank each) -> 2 banks
        pw = ctx.enter_context(tc.tile_pool(name="pw", bufs=3, space="PSUM"))
        py = ctx.enter_context(tc.tile_pool(name="py", bufs=1, space="PSUM"))

        wkq = const.tile([C + 1, 128], BF16)
        nc.sync.dma_start(wkq[:], wkq_d[:])
        wv = const.tile([C + 1, C], BF16)
        nc.sync.dma_start(wv[:], wv_d[:])

        # prefetch all x batches up front
        xps = []
        for b in range(BPC):
            xp = xpool.tile([C + 1, L], BF16)
            nc.sync.dma_start(xp[:], x_d[b])
            xps.append(xp)

        def prep_tasks(b):
            # staged emission closures computing kd/qd/vsb for batch b.
            # kd: rows 0-49 = k^T (engine-written), rows 64-113 = DMA dup.
            # qd: rows 64-113 = q^T (engine-written), rows 0-49 = DMA dup.
            # The duplicate partition range lets mm2 run 2x row-tiled.
            xp = xps[b]
            kd = kqpool.tile([128, L], BF16)
            qd = kqpool.tile([128, L], BF16)
            vsb = vpool.tile([128, NM * C], BF16)
            state = {}

            def s1_a(p):
                pkq = pw.tile([128, 1024], FP32, name="pwm")
                for j in range(2):
                    nc.tensor.matmul(
                        pkq[:, ts(j, 512)],
                        wkq[:],
                        xp[:, ts(2 * p + j, 512)],
                        start=True,
                        stop=True,
                    )
                xm = tmppool.tile([128, 1024], FP32, name="xm")
                nc.vector.tensor_scalar_min(xm[:], pkq[:], 0.0)
                state[p] = (pkq, xm)

            def s1_b(p):
                pkq, xm = state.pop(p)
                xe = tmppool.tile([128, 1024], FP32, name="xe")
                nc.scalar.activation(xe[:], xm[:], AF.Exp)
                # elu = max(x, exp(min(x,0)) - 1); k rows then q rows
                nc.vector.scalar_tensor_tensor(
                    kd[0:KQ, ts(p, 1024)], xe[0:KQ, :], -1.0, pkq[0:KQ, :],
                    OP.add, OP.max,
                )
                nc.vector.scalar_tensor_tensor(
                    qd[64 : 64 + KQ, ts(p, 1024)], xe[64 : 64 + KQ, :], -1.0,
                    pkq[64 : 64 + KQ, :], OP.add, OP.max,
                )

            def dup():
                # partition-shifted copies enabling row-tiled mm2
                nc.sync.dma_start(kd[64 : 64 + KQ, :], kd[0:KQ, :])
                nc.sync.dma_start(qd[0:KQ, :], qd[64 : 64 + KQ, :])

            def v_a():
                pv = pw.tile([128, 1024], FP32, name="pwm")
                for mm in range(NM):
                    nc.tensor.matmul(
                        pv[:, ts(mm, C)], xp[:, ts(mm, 128)], wv[:],
                        start=True, stop=True,
                    )
                state["v"] = pv

            def v_b():
                pv = state.pop("v")
                nc.scalar.activation(vsb[:], pv[:], AF.Tanh)

            tasks = [
                lambda: s1_a(0), lambda: s1_b(0),
                lambda: s1_a(1), lambda: (s1_b(1), dup()),
                v_a, v_b,
            ]
            return kd, qd, vsb, tasks

        kd, qd, vsb, tasks = prep_tasks(0)
        for t in tasks:
            t()

        for b in range(BPC):
            if b + 1 < BPC:
                kd_n, qd_n, vsb_n, tasks = prep_tasks(b + 1)
            else:
                tasks = []

            # y^T accumulators, partition-packed 2 l-chunks per PSUM bank
            pyt = [py.tile([128, 512], FP32, name=f"pyt{j}") for j in range(2)]

            def emit_mm2(m):
                # row-tiled 2x: tile (0,0) streams l-chunks 0,1 from the lower
                # partition copy; tile (64,0) streams chunks 2,3 from the upper
                # copy. The two tiles run concurrently into different banks.
                tiles = []
                for h in range(2):
                    pwm = pw.tile([128, 1024], FP32, name="pwm")
                    for jj in range(2):
                        nc.tensor.matmul(
                            pwm[:, ts(jj, 512)],
                            qd[64 * h : 64 * h + KQ, ts(m, 128)],
                            kd[64 * h : 64 * h + KQ, ts(2 * h + jj, 512)],
                            start=True,
                            stop=True,
                        )
                    tiles.append(pwm)
                return tiles

            pw2s = {0: emit_mm2(0), 1: emit_mm2(1)}
            ti = 0
            for m in range(NM):
                pw2 = pw2s.pop(m)
                et = []
                for h in range(2):
                    e = epool.tile([128, 1024], BF16)
                    nc.scalar.activation(e[:], pw2[h][:], AF.Exp, scale=scale)
                    et.append(e)
                if m + 2 < NM:
                    pw2s[m + 2] = emit_mm2(m + 2)
                # softmax denominator on DVE. bf16 partial sums keep every
                # operand 2-byte/packed so the DVE 2x perf mode can engage;
                # the DVE ALU accumulates in fp32, only the store is rounded.
                d4 = spool.tile([128, 4], BF16)
                with nc.allow_low_precision("bf16 store of softmax denom"):
                    nc.vector.reduce_sum(
                        d4[:, 0:2],
                        et[0][:].rearrange("p (a b) -> p a b", b=512),
                        axis=AX.X,
                    )
                    nc.vector.reduce_sum(
                        d4[:, 2:4],
                        et[1][:].rearrange("p (a b) -> p a b", b=512),
                        axis=AX.X,
                    )
                dsum = spool.tile([128, 1], FP32)
                nc.vector.reduce_sum(dsum[:], d4[:], axis=AX.X)
                dinv = spool.tile([128, 1], FP32)
                nc.vector.reciprocal(dinv[:], dsum[:])
                vp = vppool.tile([128, C], BF16)
                nc.gpsimd.tensor_scalar_mul(vp[:], vsb[:, ts(m, C)], dinv[:])
                # mm3 column-tiled: two concurrent matmuls per e-half, the
                # second writing PSUM partitions 64-127 (tile (0,64))
                for h in range(2):
                    nc.tensor.matmul(
                        pyt[h][0:64, :], vp[:], et[h][:, 0:512],
                        start=(m == 0), stop=(m == NM - 1),
                    )
                    # skip_group_check: the sim's zero-region tracker ignores
                    # partition bases; the low/high halves are disjoint groups
                    nc.tensor.matmul(
                        pyt[h][64:128, :], vp[:], et[h][:, 512:1024],
                        start=(m == 0), stop=(m == NM - 1),
                        skip_group_check=True,
                    )
                # interleave next-batch prep stages
                if ti < len(tasks) and m in (3, 4, 7, 8, 11, 12):
                    tasks[ti]()
                    ti += 1

            while ti < len(tasks):
                tasks[ti]()
                ti += 1

            # epilogue: psum -> sbuf -> DRAM (host does the [C,L] -> [L,C]
            # transpose). yT l-chunks: 0=yt[0:64,0:512], 1=yt[64:128,0:512],
            # 2=yt[0:64,512:1024], 3=yt[64:128,512:1024]
            yt = ytpool.tile([128, 1024], FP32)
            nc.vector.tensor_copy(yt[:, 0:512], pyt[0][:])
            nc.vector.tensor_copy(yt[:, 512:1024], pyt[1][:])
            nc.sync.dma_start(y_d[b][:, 0:512], yt[0:64, 0:512])
            nc.sync.dma_start(y_d[b][:, 512:1024], yt[64:128, 0:512])
            nc.sync.dma_start(y_d[b][:, 1024:1536], yt[0:64, 512:1024])
            nc.sync.dma_start(y_d[b][:, 1536:2048], yt[64:128, 512:1024])

            if b + 1 < BPC:
                kd, qd, vsb = kd_n, qd_n, vsb_n

    nc.finalize()
    return nc


def kernel(x, Wk, bk, Wq, bq, Wv, bv, sample_len):
    global LAST_EXEC_NS
    from concourse.bass_utils import run_bass_kernel_spmd

    scale = float(1.0 / np.sqrt(np.float64(sample_len)))
    if scale not in _cache:
        _cache[scale] = _build(scale)
    nc = _cache[scale]

    import ml_dtypes

    bf16 = ml_dtypes.bfloat16
    x = np.asarray(x, dtype=np.float32)
    ones = np.ones((B, 1, L), dtype=np.float32)
    x = np.ascontiguousarray(np.concatenate([x, ones], axis=1)).astype(bf16)
    wkq = np.zeros((C + 1, 128), dtype=np.float32)
    wkq[:, 0:KQ] = np.concatenate([Wk, bk[None, :]], axis=0)
    wkq[:, 64 : 64 + KQ] = np.concatenate([Wq, bq[None, :]], axis=0)
    wkq = wkq.astype(bf16)
    wv = np.concatenate([Wv, bv[None, :]], axis=0).astype(bf16)

    in_maps = [
        {"x": x[i * BPC : (i + 1) * BPC], "wkq": wkq, "wv": wv}
        for i in range(NCORES)
    ]
    res = run_bass_kernel_spmd(nc, in_maps, list(range(NCORES)), trace=TRACE)
    LAST_EXEC_NS = res.exec_time_ns
    yt = np.concatenate([res.results[i]["y"] for i in range(NCORES)], axis=0)
    # yt: [B, C, L] -> y: [B, L, C]
    return np.ascontiguousarray(yt.transpose(0, 2, 1))


# revision 17
# speedup vs baseline: 1.1720x; 1.1720x over previous
import numpy as np

# Problem constants (nn_MyTemporalAttention): x [32, 64, 2048], y [32, 2048, 64]
B, C, L = 32, 64, 2048
KQ = 50
NCORES = 8
BPC = B // NCORES          # batches per core
NM = L // 128              # m-chunks of 128
NL = L // 512              # l-chunks of 512

TRACE = False
LAST_EXEC_NS = None
_cache = {}


def _build(scale: float):
    import concourse.bass as bass
    import concourse.tile as tile
    from concourse import bacc, mybir
    from contextlib import ExitStack

    FP32 = mybir.dt.float32
    BF16 = mybir.dt.bfloat16
    AF = mybir.ActivationFunctionType
    AX = mybir.AxisListType
    OP = mybir.AluOpType
    ts = bass.ts

    nc = bacc.Bacc(
        "TRN2",
        target_bir_lowering=False,
        debug=False,
        enable_asserts=False,
        num_devices=NCORES,
    )
    # x arrives host-augmented with a ones row, pre-cast to bf16: [BPC, 65, L]
    x_d = nc.dram_tensor("x", [BPC, C + 1, L], BF16, kind="ExternalInput").ap()
    # wkq: [65, 128], cols 0-49 = [Wk; bk], cols 64-113 = [Wq; bq], rest zero
    # (q on a 32-aligned partition base so engine ops can address its rows)
    wkq_d = nc.dram_tensor("wkq", [C + 1, 128], BF16, kind="ExternalInput").ap()
    # wv = [Wv; bv] -> [65, 64]
    wv_d = nc.dram_tensor("wv", [C + 1, C], BF16, kind="ExternalInput").ap()
    # y stored transposed per batch: [BPC, C(=64), L]
    y_d = nc.dram_tensor("y", [BPC, C, L], FP32, kind="ExternalOutput").ap()

    with tile.TileContext(nc) as tc, ExitStack() as ctx:
        const = ctx.enter_context(tc.tile_pool(name="const", bufs=1))
        xpool = ctx.enter_context(tc.tile_pool(name="xp", bufs=BPC))
        kqpool = ctx.enter_context(tc.tile_pool(name="kq", bufs=4))
        tmppool = ctx.enter_context(tc.tile_pool(name="tmp", bufs=2))
        vpool = ctx.enter_context(tc.tile_pool(name="v", bufs=2))
        epool = ctx.enter_context(tc.tile_pool(name="e", bufs=10))
        espool = ctx.enter_context(tc.tile_pool(name="es", bufs=2))
        ytpool = ctx.enter_context(tc.tile_pool(name="yt", bufs=2))
        spool = ctx.enter_context(tc.tile_pool(name="stats", bufs=16))
        vppool = ctx.enter_context(tc.tile_pool(name="vp", bufs=6))
        # PSUM: pw = tag pwm [128,1024] (2 banks), bufs=3 -> 6 banks
        #       py = pyt0/pyt1 [128,512] (1 bank each) -> 2 banks
        pw = ctx.enter_context(tc.tile_pool(name="pw", bufs=3, space="PSUM"))
        py = ctx.enter_context(tc.tile_pool(name="py", bufs=1, space="PSUM"))

        wkq = const.tile([C + 1, 128], BF16)
        nc.sync.dma_start(wkq[:], wkq_d[:])
        wv = const.tile([C + 1, C], BF16)
        nc.sync.dma_start(wv[:], wv_d[:])

        # prefetch all x batches up front
        xps = []
        for b in range(BPC):
            xp = xpool.tile([C + 1, L], BF16)
            nc.sync.dma_start(xp[:], x_d[b])
            xps.append(xp)

        def prep_tasks(b):
            # staged emission closures computing kd/qd/vsb for batch b.
            # kd: rows 0-49 = k^T (engine-written), rows 64-113 = DMA dup.
            # qd: rows 64-113 = q^T (engine-written), rows 0-49 = DMA dup.
            # The duplicate partition range lets mm2 run 2x row-tiled.
            xp = xps[b]
            kd = kqpool.tile([128, L], BF16)
            qd = kqpool.tile([128, L], BF16)
            vsb = vpool.tile([128, NM * C], BF16)
            state = {}

            def s1_a(p):
                pkq = pw.tile([128, 1024], FP32, name="pwm")
                for j in range(2):
                    nc.tensor.matmul(
                        pkq[:, ts(j, 512)],
                        wkq[:],
                        xp[:, ts(2 * p + j, 512)],
                        start=True,
                        stop=True,
                    )
                xm = tmppool.tile([128, 1024], FP32, name="xm")
                nc.vector.tensor_scalar_min(xm[:], pkq[:], 0.0)
                state[p] = (pkq, xm)

            def s1_b(p):
                pkq, xm = state.pop(p)
                xe = tmppool.tile([128, 1024], FP32, name="xe")
                nc.scalar.activation(xe[:], xm[:], AF.Exp)
                # elu = max(x, exp(min(x,0)) - 1); k rows then q rows
                nc.vector.scalar_tensor_tensor(
                    kd[0:KQ, ts(p, 1024)], xe[0:KQ, :], -1.0, pkq[0:KQ, :],
                    OP.add, OP.max,
                )
                nc.vector.scalar_tensor_tensor(
                    qd[64 : 64 + KQ, ts(p, 1024)], xe[64 : 64 + KQ, :], -1.0,
                    pkq[64 : 64 + KQ, :], OP.add, OP.max,
                )

            def dup():
                # partition-shifted copies enabling row-tiled mm2
                nc.sync.dma_start(kd[64 : 64 + KQ, :], kd[0:KQ, :])
                nc.sync.dma_start(qd[0:KQ, :], qd[64 : 64 + KQ, :])

            def v_a():
                pv = pw.tile([128, 1024], FP32, name="pwm")
                for mm in range(NM):
                    nc.tensor.matmul(
                        pv[:, ts(mm, C)], xp[:, ts(mm, 128)], wv[:],
                        start=True, stop=True,
                    )
                state["v"] = pv

            def v_b():
                pv = state.pop("v")
                nc.scalar.activation(vsb[:], pv[:], AF.Tanh)

            tasks = [
                lambda: s1_a(0), lambda: s1_b(0),
                lambda: s1_a(1), lambda: (s1_b(1), dup()),
                v_a, v_b,
            ]
            return kd, qd, vsb, tasks

        kd, qd, vsb, tasks = prep_tasks(0)
        for t in tasks:
            t()

        for b in range(BPC):
            if b + 1 < BPC:
                kd_n, qd_n, vsb_n, tasks = prep_tasks(b + 1)
            else:
                tasks = []

            # y^T accumulators, partition-packed 2 l-chunks per PSUM bank
            pyt = [py.tile([128, 512], FP32, name=f"pyt{j}") for j in range(2)]

            def emit_mm2(m):
                # row-tiled 2x: tile (0,0) streams l-chunks 0,1 from the lower
                # partition copy; tile (64,0) streams chunks 2,3 from the upper
                # copy. The two tiles run concurrently into different banks.
                tiles = []
                for h in range(2):
                    pwm = pw.tile([128, 1024], FP32, name="pwm")
                    for jj in range(2):
                        nc.tensor.matmul(
                            pwm[:, ts(jj, 512)],
                            qd[64 * h : 64 * h + KQ, ts(m, 128)],
                            kd[64 * h : 64 * h + KQ, ts(2 * h + jj, 512)],
                            start=True,
                            stop=True,
                        )
                    tiles.append(pwm)
                return tiles

            pw2s = {0: emit_mm2(0), 1: emit_mm2(1)}
            ti = 0
            for m in range(NM):
                pw2 = pw2s.pop(m)
                d2 = spool.tile([128, 2], FP32)
                et = []
                for h in range(2):
                    e = epool.tile([128, 1024], BF16)
                    # h0 folds its softmax-denominator sum into the exp via
                    # the ACT accumulator; h1's sum runs on DVE so neither
                    # engine carries both.
                    nc.scalar.activation(
                        e[:], pw2[h][:], AF.Exp, scale=scale,
                        accum_out=(d2[:, 0:1] if h == 0 else None),
                    )
                    et.append(e)
                if m + 2 < NM:
                    pw2s[m + 2] = emit_mm2(m + 2)
                nc.vector.reduce_sum(d2[:, 1:2], et[1][:], axis=AX.X)
                dsum = spool.tile([128, 1], FP32)
                nc.vector.tensor_add(dsum[:], d2[:, 0:1], d2[:, 1:2])
                dinv = spool.tile([128, 1], FP32)
                nc.vector.reciprocal(dinv[:], dsum[:])
                vp = vppool.tile([128, C], BF16)
                nc.vector.tensor_scalar_mul(vp[:], vsb[:, ts(m, C)], dinv[:])
                # mm3 column-tiled: two concurrent matmuls per e-half, the
                # second writing PSUM partitions 64-127 (tile (0,64))
                for h in range(2):
                    nc.tensor.matmul(
                        pyt[h][0:64, :], vp[:], et[h][:, 0:512],
                        start=(m == 0), stop=(m == NM - 1),
                    )
                    # skip_group_check: the sim's zero-region tracker ignores
                    # partition bases; the low/high halves are disjoint groups
                    nc.tensor.matmul(
                        pyt[h][64:128, :], vp[:], et[h][:, 512:1024],
                        start=(m == 0), stop=(m == NM - 1),
                        skip_group_check=True,
                    )
                # interleave next-batch prep stages
                if ti < len(tasks) and m in (3, 4, 7, 8, 11, 12):
                    tasks[ti]()
                    ti += 1

            while ti < len(tasks):
                tasks[ti]()
                ti += 1

            # epilogue: psum -> sbuf -> DRAM (host does the [C,L] -> [L,C]
            # transpose). yT l-chunks: 0=yt[0:64,0:512], 1=yt[64:128,0:512],
            # 2=yt[0:64,512:1024], 3=yt[64:128,512:1024]
            yt = ytpool.tile([128, 1024], FP32)
            nc.vector.tensor_copy(yt[:, 0:512], pyt[0][:])
            nc.vector.tensor_copy(yt[:, 512:1024], pyt[1][:])
            nc.sync.dma_start(y_d[b][:, 0:512], yt[0:64, 0:512])
            nc.sync.dma_start(y_d[b][:, 512:1024], yt[64:128, 0:512])
            nc.sync.dma_start(y_d[b][:, 1024:1536], yt[0:64, 512:1024])
            nc.sync.dma_start(y_d[b][:, 1536:2048], yt[64:128, 512:1024])

            if b + 1 < BPC:
                kd, qd, vsb = kd_n, qd_n, vsb_n

    nc.finalize()
    return nc


def kernel(x, Wk, bk, Wq, bq, Wv, bv, sample_len):
    global LAST_EXEC_NS
    from concourse.bass_utils import run_bass_kernel_spmd

    scale = float(1.0 / np.sqrt(np.float64(sample_len)))
    if scale not in _cache:
        _cache[scale] = _build(scale)
    nc = _cache[scale]

    import ml_dtypes

    bf16 = ml_dtypes.bfloat16
    x = np.asarray(x, dtype=np.float32)
    ones = np.ones((B, 1, L), dtype=np.float32)
    x = np.ascontiguousarray(np.concatenate([x, ones], axis=1)).astype(bf16)
    wkq = np.zeros((C + 1, 128), dtype=np.float32)
    wkq[:, 0:KQ] = np.concatenate([Wk, bk[None, :]], axis=0)
    wkq[:, 64 : 64 + KQ] = np.concatenate([Wq, bq[None, :]], axis=0)
    wkq = wkq.astype(bf16)
    wv = np.concatenate([Wv, bv[None, :]], axis=0).astype(bf16)

    in_maps = [
        {"x": x[i * BPC : (i + 1) * BPC], "wkq": wkq, "wv": wv}
        for i in range(NCORES)
    ]
    res = run_bass_kernel_spmd(nc, in_maps, list(range(NCORES)), trace=TRACE)
    LAST_EXEC_NS = res.exec_time_ns
    yt = np.concatenate([res.results[i]["y"] for i in range(NCORES)], axis=0)
    # yt: [B, C, L] -> y: [B, L, C]
    return np.ascontiguousarray(yt.transpose(0, 2, 1))


# revision 18
# speedup vs baseline: 1.2574x; 1.0728x over previous
import numpy as np

# Problem constants (nn_MyTemporalAttention): x [32, 64, 2048], y [32, 2048, 64]
B, C, L = 32, 64, 2048
KQ = 50
NCORES = 8
BPC = B // NCORES          # batches per core
NM = L // 128              # m-chunks of 128
NL = L // 512              # l-chunks of 512

TRACE = False
LAST_EXEC_NS = None
_cache = {}


def _build(scale: float):
    import concourse.bass as bass
    import concourse.tile as tile
    from concourse import bacc, mybir
    from contextlib import ExitStack

    FP32 = mybir.dt.float32
    BF16 = mybir.dt.bfloat16
    AF = mybir.ActivationFunctionType
    AX = mybir.AxisListType
    OP = mybir.AluOpType
    ts = bass.ts

    nc = bacc.Bacc(
        "TRN2",
        target_bir_lowering=False,
        debug=False,
        enable_asserts=False,
        num_devices=NCORES,
    )
    # x arrives host-augmented with a ones row, pre-cast to bf16: [BPC, 65, L]
    x_d = nc.dram_tensor("x", [BPC, C + 1, L], BF16, kind="ExternalInput").ap()
    # wkq: [65, 128], cols 0-49 = [Wk; bk], cols 64-113 = [Wq; bq], rest zero
    # (q on a 32-aligned partition base so engine ops can address its rows)
    wkq_d = nc.dram_tensor("wkq", [C + 1, 128], BF16, kind="ExternalInput").ap()
    # wv = [Wv; bv] -> [65, 64]
    wv_d = nc.dram_tensor("wv", [C + 1, C], BF16, kind="ExternalInput").ap()
    # y stored transposed per batch: [BPC, C(=64), L]
    y_d = nc.dram_tensor("y", [BPC, C, L], FP32, kind="ExternalOutput").ap()

    with tile.TileContext(nc) as tc, ExitStack() as ctx:
        const = ctx.enter_context(tc.tile_pool(name="const", bufs=1))
        xpool = ctx.enter_context(tc.tile_pool(name="xp", bufs=BPC))
        kqpool = ctx.enter_context(tc.tile_pool(name="kq", bufs=4))
        tmppool = ctx.enter_context(tc.tile_pool(name="tmp", bufs=2))
        vpool = ctx.enter_context(tc.tile_pool(name="v", bufs=2))
        epool = ctx.enter_context(tc.tile_pool(name="e", bufs=10))
        espool = ctx.enter_context(tc.tile_pool(name="es", bufs=2))
        ytpool = ctx.enter_context(tc.tile_pool(name="yt", bufs=2))
        spool = ctx.enter_context(tc.tile_pool(name="stats", bufs=16))
        vppool = ctx.enter_context(tc.tile_pool(name="vp", bufs=6))
        # PSUM: pw = tag pwm [128,1024] (2 banks), bufs=3 -> 6 banks
        #       py = pyt0/pyt1 [128,512] (1 bank each) -> 2 banks
        pw = ctx.enter_context(tc.tile_pool(name="pw", bufs=3, space="PSUM"))
        py = ctx.enter_context(tc.tile_pool(name="py", bufs=1, space="PSUM"))

        wkq = const.tile([C + 1, 128], BF16)
        nc.sync.dma_start(wkq[:], wkq_d[:])
        wv = const.tile([C + 1, C], BF16)
        nc.sync.dma_start(wv[:], wv_d[:])

        # prefetch all x batches up front
        xps = []
        for b in range(BPC):
            xp = xpool.tile([C + 1, L], BF16)
            nc.sync.dma_start(xp[:], x_d[b])
            xps.append(xp)

        def prep_tasks(b):
            # staged emission closures computing kd/qd/vsb for batch b.
            # kd: rows 0-49 = k^T (engine-written), rows 64-113 = DMA dup.
            # qd: rows 64-113 = q^T (engine-written), rows 0-49 = DMA dup.
            # The duplicate partition range lets mm2 run 2x row-tiled.
            xp = xps[b]
            kd = kqpool.tile([128, L], BF16)
            qd = kqpool.tile([128, L], BF16)
            vsb = vpool.tile([128, NM * C], BF16)
            state = {}

            def s1_a(p):
                pkq = pw.tile([128, 1024], FP32, name="pwm")
                for j in range(2):
                    nc.tensor.matmul(
                        pkq[:, ts(j, 512)],
                        wkq[:],
                        xp[:, ts(2 * p + j, 512)],
                        start=True,
                        stop=True,
                    )
                xm = tmppool.tile([128, 1024], FP32, name="xm")
                nc.vector.tensor_scalar_min(xm[:], pkq[:], 0.0)
                state[p] = (pkq, xm)

            def s1_b(p):
                pkq, xm = state.pop(p)
                xe = tmppool.tile([128, 1024], FP32, name="xe")
                nc.scalar.activation(xe[:], xm[:], AF.Exp)
                # elu = max(x, exp(min(x,0)) - 1); k rows then q rows
                nc.vector.scalar_tensor_tensor(
                    kd[0:KQ, ts(p, 1024)], xe[0:KQ, :], -1.0, pkq[0:KQ, :],
                    OP.add, OP.max,
                )
                nc.vector.scalar_tensor_tensor(
                    qd[64 : 64 + KQ, ts(p, 1024)], xe[64 : 64 + KQ, :], -1.0,
                    pkq[64 : 64 + KQ, :], OP.add, OP.max,
                )

            def dup():
                # partition-shifted copies enabling row-tiled mm2
                nc.sync.dma_start(kd[64 : 64 + KQ, :], kd[0:KQ, :])
                nc.sync.dma_start(qd[0:KQ, :], qd[64 : 64 + KQ, :])

            def v_a():
                pv = pw.tile([128, 1024], FP32, name="pwm")
                for mm in range(NM):
                    nc.tensor.matmul(
                        pv[:, ts(mm, C)], xp[:, ts(mm, 128)], wv[:],
                        start=True, stop=True,
                    )
                state["v"] = pv

            def v_b():
                pv = state.pop("v")
                nc.scalar.activation(vsb[:], pv[:], AF.Tanh)

            tasks = [
                lambda: s1_a(0), lambda: s1_b(0),
                lambda: s1_a(1), lambda: (s1_b(1), dup()),
                v_a, v_b,
            ]
            return kd, qd, vsb, tasks

        kd, qd, vsb, tasks = prep_tasks(0)
        for t in tasks:
            t()

        for b in range(BPC):
            if b + 1 < BPC:
                kd_n, qd_n, vsb_n, tasks = prep_tasks(b + 1)
            else:
                tasks = []

            # y^T accumulators, partition-packed 2 l-chunks per PSUM bank
            pyt = [py.tile([128, 512], FP32, name=f"pyt{j}") for j in range(2)]

            def emit_mm2(m):
                # row-tiled 2x: PE tile (0,0) streams the even l-chunk from
                # the lower partition copy while tile (64,0) streams the odd
                # chunk from the upper copy. Both write the same pwm tile
                # (different banks) so they unblock together and overlap.
                tiles = []
                for h in range(2):
                    pwm = pw.tile([128, 1024], FP32, name="pwm")
                    for jj in range(2):
                        lo = 64 * jj
                        nc.tensor.matmul(
                            pwm[:, ts(jj, 512)],
                            qd[lo : lo + KQ, ts(m, 128)],
                            kd[lo : lo + KQ, ts(2 * h + jj, 512)],
                            start=True,
                            stop=True,
                        )
                    tiles.append(pwm)
                return tiles

            pw2s = {0: emit_mm2(0), 1: emit_mm2(1)}
            ti = 0
            for m in range(NM):
                pw2 = pw2s.pop(m)
                d2 = spool.tile([128, 2], FP32)
                et = []
                for h in range(2):
                    e = epool.tile([128, 1024], BF16)
                    # h0 folds its softmax-denominator sum into the exp via
                    # the ACT accumulator; h1's sum runs on DVE so neither
                    # engine carries both.
                    nc.scalar.activation(
                        e[:], pw2[h][:], AF.Exp, scale=scale,
                        accum_out=(d2[:, 0:1] if h == 0 else None),
                    )
                    et.append(e)
                if m + 2 < NM:
                    pw2s[m + 2] = emit_mm2(m + 2)
                nc.vector.reduce_sum(d2[:, 1:2], et[1][:], axis=AX.X)
                dsum = spool.tile([128, 1], FP32)
                nc.vector.tensor_add(dsum[:], d2[:, 0:1], d2[:, 1:2])
                dinv = spool.tile([128, 1], FP32)
                nc.vector.reciprocal(dinv[:], dsum[:])
                vp = vppool.tile([128, C], BF16)
                nc.vector.tensor_scalar_mul(vp[:], vsb[:, ts(m, C)], dinv[:])
                # mm3 column-tiled: two concurrent matmuls per e-half, the
                # second writing PSUM partitions 64-127 (tile (0,64))
                for h in range(2):
                    nc.tensor.matmul(
                        pyt[h][0:64, :], vp[:], et[h][:, 0:512],
                        start=(m == 0), stop=(m == NM - 1),
                    )
                    # skip_group_check: the sim's zero-region tracker ignores
                    # partition bases; the low/high halves are disjoint groups
                    nc.tensor.matmul(
                        pyt[h][64:128, :], vp[:], et[h][:, 512:1024],
                        start=(m == 0), stop=(m == NM - 1),
                        skip_group_check=True,
                    )
                # interleave next-batch prep stages
                if ti < len(tasks) and m in (3, 4, 7, 8, 11, 12):
                    tasks[ti]()
                    ti += 1

            while ti < len(tasks):
                tasks[ti]()
                ti += 1

            # epilogue: psum -> sbuf -> DRAM (host does the [C,L] -> [L,C]
            # transpose). yT l-chunks: 0=yt[0:64,0:512], 1=yt[64:128,0:512],
            # 2=yt[0:64,512:1024], 3=yt[64:128,512:1024]
            yt = ytpool.tile([128, 1024], FP32)
            nc.vector.tensor_copy(yt[:, 0:512], pyt[0][:])
            nc.vector.tensor_copy(yt[:, 512:1024], pyt[1][:])
            nc.sync.dma_start(y_d[b][:, 0:512], yt[0:64, 0:512])
            nc.sync.dma_start(y_d[b][:, 512:1024], yt[64:128, 0:512])
            nc.sync.dma_start(y_d[b][:, 1024:1536], yt[0:64, 512:1024])
            nc.sync.dma_start(y_d[b][:, 1536:2048], yt[64:128, 512:1024])

            if b + 1 < BPC:
                kd, qd, vsb = kd_n, qd_n, vsb_n

    nc.finalize()
    return nc


def kernel(x, Wk, bk, Wq, bq, Wv, bv, sample_len):
    global LAST_EXEC_NS
    from concourse.bass_utils import run_bass_kernel_spmd

    scale = float(1.0 / np.sqrt(np.float64(sample_len)))
    if scale not in _cache:
        _cache[scale] = _build(scale)
    nc = _cache[scale]

    import ml_dtypes

    bf16 = ml_dtypes.bfloat16
    x = np.asarray(x, dtype=np.float32)
    ones = np.ones((B, 1, L), dtype=np.float32)
    x = np.ascontiguousarray(np.concatenate([x, ones], axis=1)).astype(bf16)
    wkq = np.zeros((C + 1, 128), dtype=np.float32)
    wkq[:, 0:KQ] = np.concatenate([Wk, bk[None, :]], axis=0)
    wkq[:, 64 : 64 + KQ] = np.concatenate([Wq, bq[None, :]], axis=0)
    wkq = wkq.astype(bf16)
    wv = np.concatenate([Wv, bv[None, :]], axis=0).astype(bf16)

    in_maps = [
        {"x": x[i * BPC : (i + 1) * BPC], "wkq": wkq, "wv": wv}
        for i in range(NCORES)
    ]
    res = run_bass_kernel_spmd(nc, in_maps, list(range(NCORES)), trace=TRACE)
    LAST_EXEC_NS = res.exec_time_ns
    yt = np.concatenate([res.results[i]["y"] for i in range(NCORES)], axis=0)
    # yt: [B, C, L] -> y: [B, L, C]
    return np.ascontiguousarray(yt.transpose(0, 2, 1))
